# revision 12
# baseline (speedup 1.0000x reference)
"""CircleLoss Trainium2 kernel.

Full-input contract: kernel(mat, pos_mask, neg_mask) -> loss [256] f32.

Math: with block masks (cols [0,32768) positive, [32768,65536) negative)
and mat values in [-0.25, 1.25] (setup uses uniform [0,1)), the relu
terms in CircleLoss are affine:
    sp = -G*relu(OP-x)*(x-DP) = 16(x-1)^2 - 1 = 16*(x^2-2x) + 15
    sn =  G*relu(x-ON)*(x-DN) = 16 x^2    - 1
loss[b] = log1p( sum_pos exp(sp) * sum_neg exp(sn) )
        = log1p( e^14 * [sum_pos exp(16(x^2-2x))] * [sum_neg exp(16 x^2)] )

Sharding: data-parallel over B=256 rows -> 32 rows per core on 8 cores.
Each core's slice is shipped as ONE [128, 16384] fp16 tensor laid out as
partition p = 4*row + blk, free = [ neg block (8192) | pos block (8192) ].
fp16 halves HBM traffic (the memory roofline); the induced exponent error
is <= 16*2^-10 ~ 0.016 absolute, which averages out across each row's
32768-term sum -- measured loss error stays ~1e-3, well under tolerance.

Device per core: 11 pipelined DMA chunks -> 11 DVE squares (fp16, 2x rate)
-> 5 big ACT Exp passes with f32 accumulators (the exp stream at 1
elem/cycle/lane is the bottleneck; few big passes minimise the per-pass
ACTIVATE fixed cost + READ_ACCUMULATOR) -> [128, 5] partial sums to HBM.
Host folds the 4 partition-blocks per row and applies log1p (256 rows).
"""

import os
from contextlib import ExitStack

import numpy as np

B = 256
NCOLS = 65536
NPOS = 32768
N_CORES = 8
R = B // N_CORES  # 32 rows per core
GAMMA = 16.0
MARGIN = 0.25
OP, ON = 1.0 + MARGIN, -MARGIN
DP, DN = 1.0 - MARGIN, MARGIN

BLK = 4  # row-blocks per half; partition p = 4*row + blk
HALF = NPOS // BLK  # 8192 free elements per partition per half
FREE = 2 * HALF  # 16384: [neg | pos]

# Stream layout: free = [ neg 8192 | pos 8192 ], but the DMA ORDER is
# neg[0:4096], pos[0:8192], neg[4096:8192]: cheap neg squares (one 2x TT)
# bracket the stream so ACT starts early and the tail chunk's square is
# short; the pos chunks' two-op squares (TS shift + TT square) hide in
# the middle.  All chunks are 2048 units = 4 KB partition lines -- short
# lines halve DMA throughput (packet-rate bound at ~2.4KB lines).
# (start, size) in stream order, alternating between the neg half and
# the pos half: consecutive chunks land >=14KB apart per partition so
# the DMA writes never share SBUF banks with the region DVE/ACT are
# reading (adjacent-region conflicts stall the slowest SDMA engines and
# delay chunk-completion semaphores by 1-3us).  The alternation also
# interleaves cheap neg squares (one 2x TT) with two-op pos squares
# (TS shift + TT), pacing the DVE evenly.  1024-unit head chunks land
# through the DMA ramp quickly; 2048-unit (4KB-line) chunks elsewhere
# for full stream rate (shorter lines are packet-rate-bound at half
# throughput).
DMA_CHUNKS = ((0, 1024), (8192, 1024),
              (1024, 2048), (9216, 2048),
              (3072, 2048), (11264, 2048),
              (5120, 2048), (13312, 2048),
              (7168, 1024), (15360, 1024))
# ACT chunks: (start, size, is_neg); scheduled so each pass's squares
# complete before the ACT engine reaches it (near-zero starvation).
ACT_CHUNKS = ((0, 1024, True), (8192, 1024, False),
              (1024, 2048, True), (3072, 2048, True),
              (9216, 4096, False),
              (5120, 3072, True), (13312, 3072, False))
N_STATS = len(ACT_CHUNKS)

LAST = None  # BassKernelResults of the most recent device run (for test.py)

_prog_cache = {}


def _build_program():
    import concourse.mybir as mybir
    from concourse.bacc import Bacc
    from concourse.tile import TileContext

    f16 = mybir.dt.float16
    f32 = mybir.dt.float32
    Exp = mybir.ActivationFunctionType.Exp

    nc = Bacc()
    x = nc.dram_tensor("x", [128, FREE], f16, kind="ExternalInput")
    out = nc.dram_tensor("out", [128, N_STATS], f32, kind="ExternalOutput")

    with TileContext(nc) as tc, ExitStack() as ctx:
        pool = ctx.enter_context(tc.tile_pool(name="d", bufs=1))
        ppool = ctx.enter_context(tc.tile_pool(name="p", bufs=1, space="PSUM"))
        X = pool.tile([128, FREE], f16)
        U = pool.tile([128, FREE], f16)
        # exp outputs are write-only (only the accumulator matters); put
        # them in PSUM so ACT's output stream stays off the SBUF fabric
        # the DMA engines are writing through.
        E = ppool.tile([128, 4096], f32)
        stats = pool.tile([128, N_STATS], f32)

        # input stream: all triggers issue back-to-back on the Sync queue
        for off, F in DMA_CHUNKS:
            nc.sync.dma_start(out=X[:, off : off + F], in_=x[:, off : off + F])

        # squares on DVE, in stream order.
        #   neg: u = x^2    one tensor_tensor      (fp16 2x rate)
        #   pos: u = (x-1)^2 = tensor_scalar shift (4x) + tensor_tensor (2x)
        # (scalar_tensor_tensor would be one op but runs at 1x -- slower.)
        for off, F in DMA_CHUNKS:
            xs = X[:, off : off + F]
            us = U[:, off : off + F]
            if off < HALF:
                nc.vector.tensor_tensor(us, xs, xs, mybir.AluOpType.mult)
            else:
                nc.vector.tensor_scalar(
                    xs, xs, -1.0, None, mybir.AluOpType.add
                )
                nc.vector.tensor_tensor(us, xs, xs, mybir.AluOpType.mult)

        # exp + row-accumulate on ACT: few big passes over U slices
        for c, (o, F, _) in enumerate(ACT_CHUNKS):
            nc.scalar.activation(
                E[:, 0:F], U[:, o : o + F], Exp, bias=0.0, scale=GAMMA,
                accum_out=stats[:, c : c + 1],
            )

        nc.sync.dma_start(out=out[:, :], in_=stats[:])

    nc.finalize()
    return nc


def _host_reference(mat, pos_mask, neg_mask):
    """General fallback for inputs that don't match the expected structure."""
    x = mat.astype(np.float64)
    sp = -GAMMA * np.maximum(OP - x, 0.0) * (x - DP)
    sn = GAMMA * np.maximum(x - ON, 0.0) * (x - DN)
    psum = (np.exp(sp) * (pos_mask == 1)).sum(axis=1)
    nsum = (np.exp(sn) * (neg_mask == 1)).sum(axis=1)
    return np.log1p(psum * nsum).astype(np.float32)


def _structured(mat, pos_mask, neg_mask):
    if mat.shape != (B, NCOLS):
        return False
    if mat.min() < -MARGIN or mat.max() > OP:
        return False
    if not (pos_mask[:, :NPOS] == 1).all() or (pos_mask[:, NPOS:] == 1).any():
        return False
    if not (neg_mask[:, NPOS:] == 1).all() or (neg_mask[:, :NPOS] == 1).any():
        return False
    return True


def kernel(mat, pos_mask, neg_mask):
    global LAST
    mat = np.ascontiguousarray(mat, dtype=np.float32)
    if not _structured(mat, pos_mask, neg_mask):
        return _host_reference(mat, pos_mask, neg_mask)

    from concourse.bass_utils import run_bass_kernel_spmd

    if "prog" not in _prog_cache:
        _prog_cache["prog"] = _build_program()
    nc = _prog_cache["prog"]

    # per-core input: [128, 16384] fp16, partition p = 4*row + blk,
    # free = [neg 8192 | pos 8192]
    m16 = mat.astype(np.float16)
    in_maps = []
    for i in range(N_CORES):
        mc = m16[i * R : (i + 1) * R]  # [32, 65536]
        xc = np.empty((128, FREE), dtype=np.float16)
        xc[:, :HALF] = mc[:, NPOS:].reshape(128, HALF)
        xc[:, HALF:] = mc[:, :NPOS].reshape(128, HALF)
        in_maps.append({"x": xc})

    kwargs = {}
    if os.environ.get("BASS_TRACE"):
        kwargs["trace"] = True
        td = os.environ.get("KERNEL_TRACE_DIR")
        if td:
            os.makedirs(td, exist_ok=True)
            kwargs["tmpdir"] = td
    res = run_bass_kernel_spmd(nc, in_maps, core_ids=list(range(N_CORES)), **kwargs)
    LAST = res

    # host fold: blk partitions -> rows, then the final log.  Device sums
    # are exp(16 x^2) = e*exp(sn) and exp(16(x-1)^2) = e*exp(sp), so
    # loss = log1p(e^-2 * P * N).
    neg_cols = [c for c, (_, _, isn) in enumerate(ACT_CHUNKS) if isn]
    pos_cols = [c for c, (_, _, isn) in enumerate(ACT_CHUNKS) if not isn]
    losses = np.empty(B, dtype=np.float64)
    for i in range(N_CORES):
        st = res.results[i]["out"].astype(np.float64)  # [128, N_STATS]
        nsum = st[:, neg_cols].sum(axis=1).reshape(R, BLK).sum(axis=1)
        psum = st[:, pos_cols].sum(axis=1).reshape(R, BLK).sum(axis=1)
        losses[i * R : (i + 1) * R] = np.log1p(np.exp(-2.0) * psum * nsum)
    return losses.astype(np.float32)


# revision 15
# speedup vs baseline: 1.1491x; 1.1491x over previous
"""CircleLoss Trainium2 kernel.

Full-input contract: kernel(mat, pos_mask, neg_mask) -> loss [256] f32.

Math: with block masks (cols [0,32768) positive, [32768,65536) negative)
and mat values in [-0.25, 1.25] (setup uses uniform [0,1)), the relu
terms in CircleLoss are affine:
    sp = -G*relu(OP-x)*(x-DP) = 16(x-1)^2 - 1 = 16*(x^2-2x) + 15
    sn =  G*relu(x-ON)*(x-DN) = 16 x^2    - 1
loss[b] = log1p( sum_pos exp(sp) * sum_neg exp(sn) )
        = log1p( e^14 * [sum_pos exp(16(x^2-2x))] * [sum_neg exp(16 x^2)] )

Sharding: data-parallel over B=256 rows -> 32 rows per core on 8 cores.
Each core's slice is shipped as ONE [128, 16384] fp16 tensor laid out as
partition p = 4*row + blk, free = [ neg block (8192) | pos block (8192) ].
fp16 halves HBM traffic (the memory roofline); the induced exponent error
is <= 16*2^-10 ~ 0.016 absolute, which averages out across each row's
32768-term sum -- measured loss error stays ~1e-3, well under tolerance.

Device per core: 8 pipelined DMA chunks (512KB, 4KB lines) -> DVE squares
(fp16 2x tensor_tensor; pos half adds a 4x tensor_scalar x-1 shift)
-> 5 big ACT Exp passes with f32 accumulators (the exp stream at 1
elem/cycle/lane is the bottleneck; few big passes minimise the per-pass
ACTIVATE fixed cost + READ_ACCUMULATOR) -> [128, 5] partial sums to HBM.
Host folds the 4 partition-blocks per row and applies log1p (256 rows).
"""

import os
from contextlib import ExitStack

import numpy as np

B = 256
NCOLS = 65536
NPOS = 32768
N_CORES = 8
R = B // N_CORES  # 32 rows per core
GAMMA = 16.0
MARGIN = 0.25
OP, ON = 1.0 + MARGIN, -MARGIN
DP, DN = 1.0 - MARGIN, MARGIN

BLK = 4  # row-blocks per half; partition p = 4*row + blk
HALF = NPOS // BLK  # 8192 free elements per partition per half
FREE = 2 * HALF  # 16384: [neg | pos]

# Stream layout: free = [ neg 8192 | pos 8192 ], but the DMA ORDER is
# neg[0:4096], pos[0:8192], neg[4096:8192]: cheap neg squares (one 2x TT)
# bracket the stream so ACT starts early and the tail chunk's square is
# short; the pos chunks' two-op squares (TS shift + TT square) hide in
# the middle.  All chunks are 2048 units = 4 KB partition lines -- short
# lines halve DMA throughput (packet-rate bound at ~2.4KB lines).
# (start, size) in stream order: neg head (cheap one-op squares get ACT
# started early), pos middle (two-op squares hide mid-stream), neg tail
# (short final square -> short drain chain).  All chunks 2048 units =
# 4KB partition lines: shorter lines are packet-rate-bound at half DMA
# throughput; sequential offsets within each half keep HBM reads local.
DMA_CHUNKS = ((0, 2048), (2048, 2048),
              (8192, 2048), (10240, 2048),
              (12288, 2048), (14336, 2048),
              (4096, 2048), (6144, 2048))
# ACT chunks: (start, size, is_neg); scheduled so each pass's squares
# complete before the ACT engine reaches it (near-zero starvation).
ACT_CHUNKS = ((0, 2048, True), (2048, 2048, True),
              (8192, 4096, False), (12288, 4096, False),
              (4096, 4096, True))
N_STATS = len(ACT_CHUNKS)

LAST = None  # BassKernelResults of the most recent device run (for test.py)

_prog_cache = {}


def _build_program():
    import concourse.mybir as mybir
    from concourse.bacc import Bacc
    from concourse.tile import TileContext

    f16 = mybir.dt.float16
    f32 = mybir.dt.float32
    Exp = mybir.ActivationFunctionType.Exp

    nc = Bacc()
    x = nc.dram_tensor("x", [128, FREE], f16, kind="ExternalInput")
    out = nc.dram_tensor("out", [128, N_STATS], f32, kind="ExternalOutput")

    with TileContext(nc) as tc, ExitStack() as ctx:
        pool = ctx.enter_context(tc.tile_pool(name="d", bufs=1))
        X = pool.tile([128, FREE], f16)
        U = pool.tile([128, FREE], f16)
        E = pool.tile([128, 4096], f32)  # exp output scratch (accum is all we keep)
        stats = pool.tile([128, N_STATS], f32)

        # input stream: all triggers issue back-to-back on the Sync queue
        for off, F in DMA_CHUNKS:
            nc.sync.dma_start(out=X[:, off : off + F], in_=x[:, off : off + F])

        # squares on DVE, in stream order.
        #   neg: u = x^2    one tensor_tensor      (fp16 2x rate)
        #   pos: u = (x-1)^2 = tensor_scalar shift (4x) + tensor_tensor (2x)
        # (scalar_tensor_tensor would be one op but runs at 1x -- slower.)
        for off, F in DMA_CHUNKS:
            xs = X[:, off : off + F]
            us = U[:, off : off + F]
            if off < HALF:
                nc.vector.tensor_tensor(us, xs, xs, mybir.AluOpType.mult)
            else:
                nc.vector.tensor_scalar(
                    xs, xs, -1.0, None, mybir.AluOpType.add
                )
                nc.vector.tensor_tensor(us, xs, xs, mybir.AluOpType.mult)

        # exp + row-accumulate on ACT: few big passes over U slices
        for c, (o, F, _) in enumerate(ACT_CHUNKS):
            nc.scalar.activation(
                E[:, 0:F], U[:, o : o + F], Exp, bias=0.0, scale=GAMMA,
                accum_out=stats[:, c : c + 1],
            )

        nc.sync.dma_start(out=out[:, :], in_=stats[:])

    nc.finalize()
    return nc


def _host_reference(mat, pos_mask, neg_mask):
    """General fallback for inputs that don't match the expected structure."""
    x = mat.astype(np.float64)
    sp = -GAMMA * np.maximum(OP - x, 0.0) * (x - DP)
    sn = GAMMA * np.maximum(x - ON, 0.0) * (x - DN)
    psum = (np.exp(sp) * (pos_mask == 1)).sum(axis=1)
    nsum = (np.exp(sn) * (neg_mask == 1)).sum(axis=1)
    return np.log1p(psum * nsum).astype(np.float32)


def _structured(mat, pos_mask, neg_mask):
    if mat.shape != (B, NCOLS):
        return False
    if mat.min() < -MARGIN or mat.max() > OP:
        return False
    if not (pos_mask[:, :NPOS] == 1).all() or (pos_mask[:, NPOS:] == 1).any():
        return False
    if not (neg_mask[:, NPOS:] == 1).all() or (neg_mask[:, :NPOS] == 1).any():
        return False
    return True


def kernel(mat, pos_mask, neg_mask):
    global LAST
    mat = np.ascontiguousarray(mat, dtype=np.float32)
    if not _structured(mat, pos_mask, neg_mask):
        return _host_reference(mat, pos_mask, neg_mask)

    from concourse.bass_utils import run_bass_kernel_spmd

    if "prog" not in _prog_cache:
        _prog_cache["prog"] = _build_program()
    nc = _prog_cache["prog"]

    # per-core input: [128, 16384] fp16, partition p = 4*row + blk,
    # free = [neg 8192 | pos 8192]
    m16 = mat.astype(np.float16)
    in_maps = []
    for i in range(N_CORES):
        mc = m16[i * R : (i + 1) * R]  # [32, 65536]
        xc = np.empty((128, FREE), dtype=np.float16)
        xc[:, :HALF] = mc[:, NPOS:].reshape(128, HALF)
        xc[:, HALF:] = mc[:, :NPOS].reshape(128, HALF)
        in_maps.append({"x": xc})

    kwargs = {}
    if os.environ.get("BASS_TRACE"):
        kwargs["trace"] = True
        td = os.environ.get("KERNEL_TRACE_DIR")
        if td:
            os.makedirs(td, exist_ok=True)
            kwargs["tmpdir"] = td
    res = run_bass_kernel_spmd(nc, in_maps, core_ids=list(range(N_CORES)), **kwargs)
    LAST = res

    # host fold: blk partitions -> rows, then the final log.  Device sums
    # are exp(16 x^2) = e*exp(sn) and exp(16(x-1)^2) = e*exp(sp), so
    # loss = log1p(e^-2 * P * N).
    neg_cols = [c for c, (_, _, isn) in enumerate(ACT_CHUNKS) if isn]
    pos_cols = [c for c, (_, _, isn) in enumerate(ACT_CHUNKS) if not isn]
    losses = np.empty(B, dtype=np.float64)
    for i in range(N_CORES):
        st = res.results[i]["out"].astype(np.float64)  # [128, N_STATS]
        nsum = st[:, neg_cols].sum(axis=1).reshape(R, BLK).sum(axis=1)
        psum = st[:, pos_cols].sum(axis=1).reshape(R, BLK).sum(axis=1)
        losses[i * R : (i + 1) * R] = np.log1p(np.exp(-2.0) * psum * nsum)
    return losses.astype(np.float32)
